# revision 1
# baseline (speedup 1.0000x reference)
"""AdditiveAttention TRN2 kernel v5 — sin-basis scores, binade-mask range
reduction with PE-folded shift.

Same math as v4 (tanh ~= sum_m c_m sin(w_m s), scores factorized into 2M
rank-128 matmuls), with the range-reduction pipeline compressed:

  PE   : p24 = (w_m/2pi)*x + 24.0    (f32r proj + rank-1 ones-row; 24.0
         is exact in f32r, so p24 sits in the [16,32) binade exactly)
  DVE  : m_sin = bits(p24) & 0x7FFFF           (PSUM -> SBUF, int view)
         m_cos = (m_sin + 2^17) & 0x7FFFF      (pi/2 phase = 2^17 units)
  ACT  : basis = Sin(m * 2pi/2^19 - pi) = -sin(w x [+ pi/2])  (bf16)

Signs cancel in the q*k products.  Tail uses PE transposes (bf16,
1 cyc/row) into freed PSUM instead of serialized DMA transposes.
"""

import math

import ml_dtypes
import numpy as np

from concourse import bacc, mybir
from concourse import tile
from concourse.bass_utils import run_bass_kernel_spmd

B, LQ, LK, QS, KS, H, VS = 8, 256, 1024, 256, 256, 128, 256
F32 = mybir.dt.float32
F32R = mybir.dt.float32r
I32 = mybir.dt.int32
BF16 = mybir.dt.bfloat16

W_FIT = [0.0822537725, -0.298217301, -0.142006636, 0.7778114887,
         1.2988701126, 1.8225811398, 1.1451769858, 2.3609781773,
         3.4643752598, 2.9094341665, 4.0118596954]
C_FIT = [0.2297757049, -0.8389809546, -0.4004822335, 0.3248009122,
         0.1335364513, 0.0610199843, 0.0074227805, 0.026674116,
         0.0048070768, 0.0114453089, 0.0018884206]
M = len(W_FIT)

SCALE_SIN = 2.0 * math.pi / (1 << 19)
QCOS = 1 << 17          # pi/2 phase in 19-bit frac units
FMASK = 0x7FFFF

_CACHE: dict = {}


def _build():
    nc = bacc.Bacc("TRN2", target_bir_lowering=False, debug=False)
    qTd = nc.declare_dram_parameter("qTd", [QS, LQ], F32R, isOutput=False)
    kTd = nc.declare_dram_parameter("kTd", [KS, LK], F32R, isOutput=False)
    wqm = nc.declare_dram_parameter("wqm", [QS, M, H], F32R, isOutput=False)
    wkm = nc.declare_dram_parameter("wkm", [KS, M, H], F32R, isOutput=False)
    ones = nc.declare_dram_parameter("ones", [1, 512], F32R, isOutput=False)
    c24 = nc.declare_dram_parameter("c24", [1, H], F32R, isOutput=False)
    cw = nc.declare_dram_parameter("cw", [H, M], F32, isOutput=False)
    negpi = nc.declare_dram_parameter("negpi", [H, 1], F32, isOutput=False)
    ident = nc.declare_dram_parameter("ident", [128, 128], BF16, isOutput=False)
    vals = nc.declare_dram_parameter("vals", [LK, VS + 1], BF16, isOutput=False)
    out = nc.declare_dram_parameter("out", [LQ, VS], F32, isOutput=True)

    NKC = LK // 128
    SIN = mybir.ActivationFunctionType.Sin
    EXP = mybir.ActivationFunctionType.Exp
    AND = mybir.AluOpType.bitwise_and
    ADD = mybir.AluOpType.add

    with tile.TileContext(nc) as tc:
        with (
            tc.tile_pool(name="const", bufs=1) as cpool,
            tc.tile_pool(name="msk", bufs=3) as mpool,
            tc.tile_pool(name="basis", bufs=3) as bpool,
            tc.tile_pool(name="exps", bufs=2) as epool,
            tc.tile_pool(name="expt", bufs=2) as etpool,
            tc.tile_pool(name="outs", bufs=2) as opool,
            tc.tile_pool(name="scal", bufs=2) as spool,
            tc.tile_pool(name="ps_k", bufs=2, space="PSUM") as ps_k,
            tc.tile_pool(name="ps_sc", bufs=4, space="PSUM") as ps_sc,
        ):
            kTd_sb = cpool.tile([128, 2, LK], F32R)
            qTd_sb = cpool.tile([128, 2, LQ], F32R)
            wkm_sb = cpool.tile([128, 2, M, H], F32R)
            wqm_sb = cpool.tile([128, 2, M, H], F32R)
            ones_sb = cpool.tile([1, 512], F32R)
            c24_sb = cpool.tile([1, H], F32R)
            cw_sb = cpool.tile([128, M], F32)
            negpi_sb = cpool.tile([128, 1], F32)
            ident_sb = cpool.tile([128, 128], BF16)
            vals_sb = cpool.tile([128, NKC, VS + 1], BF16)
            for d in range(2):
                nc.sync.dma_start(out=kTd_sb[:, d, :], in_=kTd[128 * d:128 * (d + 1), :])
                nc.sync.dma_start(out=qTd_sb[:, d, :], in_=qTd[128 * d:128 * (d + 1), :])
                nc.sync.dma_start(out=wkm_sb[:, d], in_=wkm[128 * d:128 * (d + 1)])
                nc.sync.dma_start(out=wqm_sb[:, d], in_=wqm[128 * d:128 * (d + 1)])
            nc.sync.dma_start(out=ones_sb[:], in_=ones[:])
            nc.sync.dma_start(out=c24_sb[:], in_=c24[:])
            nc.sync.dma_start(out=cw_sb[:], in_=cw[:])
            nc.sync.dma_start(out=negpi_sb[:], in_=negpi[:])
            nc.sync.dma_start(out=ident_sb[:], in_=ident[:])
            for c in range(NKC):
                nc.sync.dma_start(out=vals_sb[:, c, :], in_=vals[128 * c:128 * (c + 1), :])

            sc = [[ps_sc.tile([128, 512], F32, tag="ps_sc", name=f"sc{qb}{hf}")
                   for hf in range(2)] for qb in range(2)]

            # ---- q-side prepass: all M terms into SBUF ----
            qsw_all = cpool.tile([128, M, LQ], BF16)
            qcw_all = cpool.tile([128, M, LQ], BF16)
            for m in range(M):
                qps = ps_k.tile([128, 256], F32, tag="ps_k", name=f"qps{m}")
                for d in range(2):
                    nc.tensor.matmul(qps[:], wqm_sb[:, d, m, :], qTd_sb[:, d, :],
                                     start=(d == 0), stop=False)
                nc.tensor.matmul(qps[:], c24_sb[:], ones_sb[:, 0:256],
                                 start=False, stop=True)
                m_q = mpool.tile([128, 2, LQ], I32, tag="m_q")
                t_q = mpool.tile([128, LQ], I32, tag="t_q")
                nc.vector.tensor_scalar(m_q[:, 0, :], qps[:].bitcast(I32),
                                        FMASK, None, AND)
                nc.vector.tensor_scalar(t_q[:], m_q[:, 0, :], QCOS, None, ADD)
                nc.vector.tensor_scalar(m_q[:, 1, :], t_q[:], FMASK, None, AND)
                bas_q = bpool.tile([128, 2, LQ], BF16, tag="bas_q")
                nc.scalar.activation(bas_q[:], m_q[:], SIN, scale=SCALE_SIN,
                                     bias=negpi_sb[:])
                nc.vector.tensor_scalar_mul(qsw_all[:, m, :], bas_q[:, 0, :],
                                            cw_sb[:, m:m + 1])
                nc.vector.tensor_scalar_mul(qcw_all[:, m, :], bas_q[:, 1, :],
                                            cw_sb[:, m:m + 1])

            # ---- k-side main loop ----
            for m in range(M):
                kps = ps_k.tile([128, 2, 512], F32, tag="ps_k", name=f"kps{m}")
                for half in range(2):
                    for d in range(2):
                        nc.tensor.matmul(
                            kps[:, half], wkm_sb[:, d, m, :],
                            kTd_sb[:, d, 512 * half:512 * (half + 1)],
                            start=(d == 0), stop=False)
                    nc.tensor.matmul(kps[:, half], c24_sb[:], ones_sb[:],
                                     start=False, stop=True)
                m_k = mpool.tile([128, 2, LK], I32, tag="m_k")
                t_k = mpool.tile([128, LK], I32, tag="t_k")
                nc.vector.tensor_scalar(m_k[:, 0, :], kps[:, :, :].bitcast(I32),
                                        FMASK, None, AND)
                nc.vector.tensor_scalar(t_k[:], m_k[:, 0, :], QCOS, None, ADD)
                nc.vector.tensor_scalar(m_k[:, 1, :], t_k[:], FMASK, None, AND)
                bas_k = bpool.tile([128, 2, LK], BF16, tag="bas_k")
                nc.scalar.activation(bas_k[:], m_k[:], SIN, scale=SCALE_SIN,
                                     bias=negpi_sb[:])

                # scores += qsw^T kc + qcw^T ks  (PE, bf16; signs cancel)
                for qb in range(2):
                    for half in range(2):
                        nc.tensor.matmul(
                            sc[qb][half][:],
                            qsw_all[:, m, 128 * qb:128 * (qb + 1)],
                            bas_k[:, 1, 512 * half:512 * (half + 1)],
                            start=(m == 0), stop=False)
                        nc.tensor.matmul(
                            sc[qb][half][:],
                            qcw_all[:, m, 128 * qb:128 * (qb + 1)],
                            bas_k[:, 0, 512 * half:512 * (half + 1)],
                            start=False, stop=(m == M - 1))

            for qb in range(2):
                expS = epool.tile([128, LK], BF16, tag="exps")
                for half in range(2):
                    nc.scalar.activation(expS[:, 512 * half:512 * (half + 1)],
                                         sc[qb][half][:], EXP)
                # attn^T via PE transpose (bf16) into the freed ps_k banks
                expT = etpool.tile([128, NKC, 128], BF16, tag="expt")
                for c in range(NKC):
                    tp = ps_k.tile([128, 128], BF16, tag="ps_k", name=f"tp{qb}{c}")
                    nc.tensor.transpose(tp[:], expS[:, 128 * c:128 * (c + 1)],
                                        ident_sb[:])
                    nc.vector.tensor_copy(expT[:, c, :], tp[:])
                av = ps_k.tile([128, VS + 1], F32, tag="ps_k")
                for c in range(NKC):
                    nc.tensor.matmul(av[:], expT[:, c, :], vals_sb[:, c, :],
                                     start=(c == 0), stop=(c == NKC - 1))
                r = spool.tile([128, 1], F32, tag="scal")
                nc.vector.reciprocal(r[:], av[:, VS:VS + 1])
                o_sb = opool.tile([128, VS], F32, tag="outs")
                nc.vector.tensor_scalar_mul(o_sb[:], av[:, 0:VS], r[:])
                nc.sync.dma_start(out=out[qb * 128:(qb + 1) * 128, :], in_=o_sb[:])

    nc.compile()
    return nc


def _make_in_maps(inputs) -> list[dict]:
    queries = np.ascontiguousarray(np.asarray(inputs["queries"], dtype=np.float32))
    key = np.ascontiguousarray(np.asarray(inputs["key"], dtype=np.float32))
    value = np.ascontiguousarray(np.asarray(inputs["value"], dtype=np.float32))
    vl = np.asarray(inputs["valid_length"], dtype=np.int32)
    W_q = np.asarray(inputs["W_q"], dtype=np.float32)
    W_k = np.asarray(inputs["W_k"], dtype=np.float32)
    W_v = np.asarray(inputs["W_v"], dtype=np.float32)

    wfit = np.asarray(W_FIT, np.float32)
    cfit = np.asarray(C_FIT, np.float32)
    s = wfit / (2.0 * math.pi)
    wqm = np.ascontiguousarray((W_q[:, None, :] * s[None, :, None]).astype(np.float32))
    wkm = np.ascontiguousarray((W_k[:, None, :] * s[None, :, None]).astype(np.float32))
    cw = np.ascontiguousarray((W_v[:, None] * cfit[None, :]).astype(np.float32))
    negpi = np.full((H, 1), -math.pi, np.float32)
    ones = np.ones((1, 512), np.float32)
    c24 = np.full((1, H), 24.0, np.float32)
    ident = np.eye(128, dtype=ml_dtypes.bfloat16)

    in_maps = []
    for b in range(B):
        v = max(int(vl[b]), 0)
        vals = np.zeros((LK, VS + 1), dtype=np.float32)
        vals[:v, :VS] = value[b, :v]
        vals[:v, VS] = 1.0
        vals = vals.astype(ml_dtypes.bfloat16)
        in_maps.append({
            "qTd": np.ascontiguousarray(queries[b].T),
            "kTd": np.ascontiguousarray(key[b].T),
            "wqm": wqm, "wkm": wkm, "ones": ones, "c24": c24,
            "cw": cw, "negpi": negpi, "ident": ident,
            "vals": vals,
        })
    return in_maps


def _postprocess(res, inputs) -> np.ndarray:
    value = np.asarray(inputs["value"], dtype=np.float32)
    vl = np.asarray(inputs["valid_length"], dtype=np.int32)
    out = np.stack([np.asarray(res.results[i]["out"]) for i in range(B)], axis=0)
    for b in range(B):
        if int(vl[b]) <= 0:
            out[b] = value[b].mean(axis=0, keepdims=True)
    return out.astype(np.float32)


def kernel(**inputs) -> np.ndarray:
    if "nc" not in _CACHE:
        _CACHE["nc"] = _build()
    nc = _CACHE["nc"]
    in_maps = _make_in_maps(inputs)
    res = run_bass_kernel_spmd(nc, in_maps, core_ids=list(range(B)))
    return _postprocess(res, inputs)



# revision 8
# speedup vs baseline: 2.0875x; 2.0875x over previous
"""AdditiveAttention TRN2 kernel v6 — base projections + per-m scaled
range reduction.

v5 recomputed the q/k projections per sin-basis term m (f32r matmuls with
pre-scaled weight copies): 99 matmul-us on the PE pipe. v6 computes the
base projections ONCE in bf16 (6 matmuls), copies them to SBUF, and
applies the per-m scale s_m = w_m/2pi plus the +24.0 binade offset as a
DVE fused mult+add. Range reduction per m:

  DVE  : p48_s = (w_m/8pi)*x + 48.0     (f32; binade [32,64): the low 16
  DVE  : p48_c = (w_m/8pi)*x + 48.0625   mantissa bits ARE the phase of
                                         w_m*x in 2^16 units; +0.0625 is
                                         the +pi/2 cos shift)
  ACT  : bas   = Sin(lo16 * 2pi/2^16 - pi) -> bf16; the Sin activation
         reads the low u16 of each f32 directly via a strided bitcast
         view, so no mask/AND instructions exist at all.
  ACT  : qw    = bas_q * cw[m]         (Copy-activation, per-part scale)
  PE   : sc   += qsw^T kc + qcw^T ks   (bf16, 8x 512-col matmuls)

q (256) and k (1024) columns ride together in [128, 1280]-wide ops to
amortize instruction overheads. Fit uses M=5 terms (end-to-end rel err
~4e-3, dominated by bf16 quantization, not the fit).
"""

import math

import ml_dtypes
import numpy as np

from concourse import bacc, mybir
from concourse import tile
from concourse.bass_utils import run_bass_kernel_spmd

B, LQ, LK, QS, KS, H, VS = 8, 256, 1024, 256, 256, 128, 256
F32 = mybir.dt.float32
I32 = mybir.dt.int32
BF16 = mybir.dt.bfloat16

W_FIT = [0.873796, 0.28935, 1.465455, 2.101898, 3.067597]
C_FIT = [0.321106, 1.231298, 0.118058, 0.055618, 0.019224]
M = len(W_FIT)

SCALE_SIN = 2.0 * math.pi / (1 << 16)
NKC = LK // 128         # 8 key chunks of 128
W = LK + LQ             # 1280: k columns then q columns

_CACHE: dict = {}


def _build():
    nc = bacc.Bacc("TRN2", target_bir_lowering=False, debug=False)
    kTd = nc.declare_dram_parameter("kTd", [KS, LK], BF16, isOutput=False)
    qTd = nc.declare_dram_parameter("qTd", [QS, LQ], BF16, isOutput=False)
    wk = nc.declare_dram_parameter("wk", [KS, H], BF16, isOutput=False)
    wq = nc.declare_dram_parameter("wq", [QS, H], BF16, isOutput=False)
    cw = nc.declare_dram_parameter("cw", [H, M], F32, isOutput=False)
    negpi = nc.declare_dram_parameter("negpi", [H, 1], F32, isOutput=False)
    ident = nc.declare_dram_parameter("ident", [128, 128], BF16, isOutput=False)
    vals = nc.declare_dram_parameter("vals", [LK, VS + 1], BF16, isOutput=False)
    out = nc.declare_dram_parameter("out", [LQ, VS], F32, isOutput=True)

    SIN = mybir.ActivationFunctionType.Sin
    EXP = mybir.ActivationFunctionType.Exp
    COPY = mybir.ActivationFunctionType.Copy
    ADD = mybir.AluOpType.add
    MULT = mybir.AluOpType.mult
    U16 = mybir.dt.uint16

    s_scale = [w / (8.0 * math.pi) for w in W_FIT]

    with tile.TileContext(nc) as tc:
        with (
            tc.tile_pool(name="const", bufs=1) as cpool,
            tc.tile_pool(name="p24p", bufs=3) as p24p,
            tc.tile_pool(name="bp", bufs=3) as bp,
            tc.tile_pool(name="qwp", bufs=3) as qwp,
            tc.tile_pool(name="ep", bufs=2) as ep,
            tc.tile_pool(name="etp", bufs=2) as etp,
            tc.tile_pool(name="op", bufs=2) as op,
            tc.tile_pool(name="sp", bufs=2) as sp,
            tc.tile_pool(name="ps_sc", bufs=1, space="PSUM") as ps_sc,
        ):
            kTd_sb = cpool.tile([128, 2, LK], BF16)
            qTd_sb = cpool.tile([128, 2, LQ], BF16)
            wk_sb = cpool.tile([128, 2, H], BF16)
            wq_sb = cpool.tile([128, 2, H], BF16)
            cw_sb = cpool.tile([128, M], F32)
            negpi_sb = cpool.tile([128, 1], F32)
            ident_sb = cpool.tile([128, 128], BF16)
            vals_sb = cpool.tile([128, NKC, VS + 1], BF16)
            base_sb = cpool.tile([128, W], F32)

            for d in range(2):
                nc.sync.dma_start(out=wk_sb[:, d, :], in_=wk[128 * d:128 * (d + 1), :])
                nc.sync.dma_start(out=kTd_sb[:, d, :], in_=kTd[128 * d:128 * (d + 1), :])
                nc.sync.dma_start(out=wq_sb[:, d, :], in_=wq[128 * d:128 * (d + 1), :])
                nc.sync.dma_start(out=qTd_sb[:, d, :], in_=qTd[128 * d:128 * (d + 1), :])
            nc.sync.dma_start(out=cw_sb[:], in_=cw[:])
            nc.sync.dma_start(out=negpi_sb[:], in_=negpi[:])
            nc.sync.dma_start(out=ident_sb[:], in_=ident[:])
            for c in range(NKC):
                nc.sync.dma_start(out=vals_sb[:, c, :], in_=vals[128 * c:128 * (c + 1), :])

            # sc[qb]: [128, 1024] f32 = 2 PSUM banks; matmuls write 512-col
            # halves (bank-aligned), exp reads the full 1024 in one call.
            sc = [ps_sc.tile([128, LK], F32, tag=f"sc{qb}", name=f"sc{qb}")
                  for qb in range(2)]

            with tc.tile_pool(name="ps_base", bufs=1, space="PSUM") as ps_base:
                base = ps_base.tile([128, W], F32, tag="base")
                # khT: [h, k] accumulated over the two 128-row halves of KS
                for half in range(2):
                    for d in range(2):
                        nc.tensor.matmul(
                            base[:, 512 * half:512 * (half + 1)],
                            wk_sb[:, d, :],
                            kTd_sb[:, d, 512 * half:512 * (half + 1)],
                            start=(d == 0), stop=(d == 1))
                for d in range(2):
                    nc.tensor.matmul(base[:, LK:W], wq_sb[:, d, :],
                                     qTd_sb[:, d, :],
                                     start=(d == 0), stop=(d == 1))
                nc.scalar.copy(base_sb[:], base[:])

                for m in range(M):
                    # sin in row 0, cos in row 1 (bias +1/16 = +pi/2 phase)
                    p48 = p24p.tile([128, 2, W], F32, tag="p48")
                    nc.vector.tensor_scalar(p48[:, 0, :], base_sb[:],
                                            float(s_scale[m]), 48.0, MULT, ADD)
                    nc.vector.tensor_scalar(p48[:, 1, :], base_sb[:],
                                            float(s_scale[m]), 48.0625, MULT, ADD)
                    bas = bp.tile([128, 2, W], BF16, tag="bas")
                    nc.scalar.activation(bas[:], p48[:].bitcast(U16)[:, :, 0::2],
                                         SIN, scale=SCALE_SIN,
                                         bias=negpi_sb[:])
                    # qw[:,0] = cw*sin_q pairs with cos_k; qw[:,1] = cw*cos_q
                    qw = qwp.tile([128, 2, LQ], BF16, tag="qw")
                    nc.scalar.activation(qw[:], bas[:, :, LK:W], COPY,
                                         scale=cw_sb[:, m:m + 1])
                    for qb in range(2):
                        for half in range(2):
                            nc.tensor.matmul(
                                sc[qb][:, 512 * half:512 * (half + 1)],
                                qw[:, 0, 128 * qb:128 * (qb + 1)],
                                bas[:, 1, 512 * half:512 * (half + 1)],
                                start=(m == 0), stop=False)
                            nc.tensor.matmul(
                                sc[qb][:, 512 * half:512 * (half + 1)],
                                qw[:, 1, 128 * qb:128 * (qb + 1)],
                                bas[:, 0, 512 * half:512 * (half + 1)],
                                start=False, stop=(m == M - 1))

            with tc.tile_pool(name="ps_tail", bufs=2, space="PSUM") as ps_tail:
                expS = [None, None]
                for qb in range(2):
                    expS[qb] = ep.tile([128, LK], BF16, tag="exps",
                                       name=f"expS{qb}")
                    nc.scalar.activation(expS[qb][:], sc[qb][:], EXP)
                for qb in range(2):
                    expT = etp.tile([128, NKC, 128], BF16, tag="expt")
                    for c in range(NKC):
                        tp = ps_tail.tile([128, 128], BF16, tag="tp",
                                          name=f"tp{qb}{c}")
                        nc.tensor.transpose(tp[:],
                                            expS[qb][:, 128 * c:128 * (c + 1)],
                                            ident_sb[:])
                        nc.vector.tensor_copy(expT[:, c, :], tp[:])
                    av = ps_tail.tile([128, VS + 1], F32, tag="av",
                                      name=f"av{qb}")
                    for c in range(NKC):
                        nc.tensor.matmul(av[:], expT[:, c, :], vals_sb[:, c, :],
                                         start=(c == 0), stop=(c == NKC - 1))
                    r = sp.tile([128, 1], F32, tag="scal")
                    nc.vector.reciprocal(r[:], av[:, VS:VS + 1])
                    o_sb = op.tile([128, VS], F32, tag="outs")
                    nc.vector.tensor_scalar_mul(o_sb[:], av[:, 0:VS], r[:])
                    nc.sync.dma_start(out=out[qb * 128:(qb + 1) * 128, :],
                                      in_=o_sb[:])

    nc.compile()
    return nc


def _make_in_maps(inputs) -> list[dict]:
    queries = np.asarray(inputs["queries"], dtype=np.float32)
    key = np.asarray(inputs["key"], dtype=np.float32)
    value = np.asarray(inputs["value"], dtype=np.float32)
    vl = np.asarray(inputs["valid_length"], dtype=np.int32)
    W_q = np.asarray(inputs["W_q"], dtype=np.float32)
    W_k = np.asarray(inputs["W_k"], dtype=np.float32)
    W_v = np.asarray(inputs["W_v"], dtype=np.float32)

    cfit = np.asarray(C_FIT, np.float32)
    cwm = np.ascontiguousarray((W_v[:, None] * cfit[None, :]).astype(np.float32))
    negpi = np.full((H, 1), -math.pi, np.float32)
    ident = np.eye(128, dtype=ml_dtypes.bfloat16)
    wk_b = np.ascontiguousarray(W_k.astype(ml_dtypes.bfloat16))
    wq_b = np.ascontiguousarray(W_q.astype(ml_dtypes.bfloat16))

    in_maps = []
    for b in range(B):
        v = max(int(vl[b]), 0)
        vals = np.zeros((LK, VS + 1), dtype=np.float32)
        vals[:v, :VS] = value[b, :v]
        vals[:v, VS] = 1.0
        vals = vals.astype(ml_dtypes.bfloat16)
        in_maps.append({
            "kTd": np.ascontiguousarray(key[b].T.astype(ml_dtypes.bfloat16)),
            "qTd": np.ascontiguousarray(queries[b].T.astype(ml_dtypes.bfloat16)),
            "wk": wk_b, "wq": wq_b, "cw": cwm,
            "negpi": negpi, "ident": ident, "vals": vals,
        })
    return in_maps


def _postprocess(res, inputs) -> np.ndarray:
    value = np.asarray(inputs["value"], dtype=np.float32)
    vl = np.asarray(inputs["valid_length"], dtype=np.int32)
    out = np.stack([np.asarray(res.results[i]["out"]) for i in range(B)], axis=0)
    for b in range(B):
        if int(vl[b]) <= 0:
            out[b] = value[b].mean(axis=0, keepdims=True)
    return out.astype(np.float32)


def kernel(**inputs) -> np.ndarray:
    if "nc" not in _CACHE:
        _CACHE["nc"] = _build()
    nc = _CACHE["nc"]
    in_maps = _make_in_maps(inputs)
    res = run_bass_kernel_spmd(nc, in_maps, core_ids=list(range(B)))
    return _postprocess(res, inputs)


# revision 9
# speedup vs baseline: 2.1822x; 1.0453x over previous
"""AdditiveAttention TRN2 kernel v7 — sin-basis scores via low-u16 binade
phase extraction.

scores[q,k] = sum_h W_v[h] tanh(qh+kh) with tanh(s) ~= sum_m c_m sin(w_m s)
factorized through sin(a+b) = sin a cos b + cos a sin b into 2M rank-128
bf16 matmuls. Per-m pipeline:

  DVE  : p48_s = (w_m/8pi)*x + 48.0     f32 in binade [32,64): the low 16
  DVE  : p48_c = (w_m/8pi)*x + 48.0625  mantissa bits ARE the phase of
                                        w_m*x in 2^16 units (+1/16 value
                                        = +pi/2 phase for the cos row)
  ACT  : bas = Sin(lo16 * 2pi/2^16 - pi) -> bf16, reading the low u16 of
         each f32 via a strided bitcast view (no mask instructions)
  DVE  : qw  = bas_q * cw[m]            (per-partition scalar multiply)
  PE   : sc += qsw^T kc + qcw^T ks      (8x 512-col bf16 matmuls)

The base projections x = [W_k^T k^T | W_q^T q^T] are computed once in
bf16 (6 matmuls) and stay in PSUM; the per-m fmas read PSUM directly.
q (256) and k (1024) columns ride together in [128, 1280]-wide ops.
Emission is software-pipelined (fma pair of iteration m before qw/scores
of m-1) so no engine stalls behind a cross-engine dependency in-order.
Dummy Exp/Sin activations at t~0 pre-load both activation tables into
the two resident slots, keeping table loads off the critical path.
"""

import math

import ml_dtypes
import numpy as np

from concourse import bacc, mybir
from concourse import tile
from concourse.bass_utils import run_bass_kernel_spmd

B, LQ, LK, QS, KS, H, VS = 8, 256, 1024, 256, 256, 128, 256
F32 = mybir.dt.float32
BF16 = mybir.dt.bfloat16

W_FIT = [0.873796, 0.28935, 1.465455, 2.101898, 3.067597]
C_FIT = [0.321106, 1.231298, 0.118058, 0.055618, 0.019224]
M = len(W_FIT)

SCALE_SIN = 2.0 * math.pi / (1 << 16)
NKC = LK // 128         # 8 key chunks of 128
W = LK + LQ             # 1280: k columns then q columns

_CACHE: dict = {}


def _build():
    nc = bacc.Bacc("TRN2", target_bir_lowering=False, debug=False)
    kTd = nc.declare_dram_parameter("kTd", [128, 2, LK], BF16, isOutput=False)
    qTd = nc.declare_dram_parameter("qTd", [128, 2, LQ], BF16, isOutput=False)
    wk = nc.declare_dram_parameter("wk", [128, 2, H], BF16, isOutput=False)
    wq = nc.declare_dram_parameter("wq", [128, 2, H], BF16, isOutput=False)
    cw = nc.declare_dram_parameter("cw", [H, M], F32, isOutput=False)
    negpi = nc.declare_dram_parameter("negpi", [H, 1], F32, isOutput=False)
    ident = nc.declare_dram_parameter("ident", [128, 128], BF16, isOutput=False)
    vals = nc.declare_dram_parameter("vals", [128, NKC, VS + 1], BF16,
                                     isOutput=False)
    out = nc.declare_dram_parameter("out", [LQ, VS], F32, isOutput=True)

    SIN = mybir.ActivationFunctionType.Sin
    EXP = mybir.ActivationFunctionType.Exp
    ADD = mybir.AluOpType.add
    MULT = mybir.AluOpType.mult
    U16 = mybir.dt.uint16

    s_scale = [w / (8.0 * math.pi) for w in W_FIT]

    with tile.TileContext(nc) as tc:
        with (
            tc.tile_pool(name="const", bufs=1) as cpool,
            tc.tile_pool(name="p48p", bufs=3) as p48p,
            tc.tile_pool(name="bp", bufs=3) as bp,
            tc.tile_pool(name="qwp", bufs=3) as qwp,
            tc.tile_pool(name="ep", bufs=2) as ep,
            tc.tile_pool(name="etp", bufs=2) as etp,
            tc.tile_pool(name="op", bufs=2) as op,
            tc.tile_pool(name="sp", bufs=2) as sp,
            tc.tile_pool(name="ps_sc", bufs=1, space="PSUM") as ps_sc,
        ):
            qTd_sb = cpool.tile([128, 2, LQ], BF16)
            wq_sb = cpool.tile([128, 2, H], BF16)
            kTd_sb = cpool.tile([128, 2, LK], BF16)
            wk_sb = cpool.tile([128, 2, H], BF16)
            cw_sb = cpool.tile([128, M], F32)
            negpi_sb = cpool.tile([128, 1], F32)
            ident_sb = cpool.tile([128, 128], BF16)
            vals_sb = cpool.tile([128, NKC, VS + 1], BF16)
            dumm = cpool.tile([128, 1], F32)
            dumo = cpool.tile([128, 2], F32)

            # table pre-loads: Exp and Sin live in different act-func sets;
            # touching both on a dummy at t~0 pulls both tables into the two
            # resident slots while the DMAs run.
            nc.gpsimd.memset(dumm[:], 0.0)
            nc.scalar.activation(dumo[:, 0:1], dumm[:], EXP)
            nc.scalar.activation(dumo[:, 1:2], dumm[:], SIN)

            # critical-path DMAs on the SP queue, tail data on the Pool queue
            nc.sync.dma_start(out=wq_sb[:], in_=wq[:])
            nc.sync.dma_start(out=qTd_sb[:], in_=qTd[:])
            nc.sync.dma_start(out=wk_sb[:], in_=wk[:])
            for d in range(2):
                nc.sync.dma_start(out=kTd_sb[:, d, :], in_=kTd[:, d, :])
            nc.gpsimd.dma_start(out=negpi_sb[:], in_=negpi[:])
            nc.gpsimd.dma_start(out=cw_sb[:], in_=cw[:])
            nc.gpsimd.dma_start(out=ident_sb[:], in_=ident[:])
            nc.gpsimd.dma_start(out=vals_sb[:], in_=vals[:])

            # sc[qb]: [128, 1024] f32 = 2 PSUM banks; matmuls write 512-col
            # halves (bank-aligned), exp reads the full 1024 in one call.
            sc = [ps_sc.tile([128, LK], F32, tag=f"sc{qb}", name=f"sc{qb}")
                  for qb in range(2)]

            with tc.tile_pool(name="ps_base", bufs=1, space="PSUM") as ps_base:
                base = ps_base.tile([128, W], F32, tag="base")
                # base = [khT | qhT]: q first (small DMAs land first),
                # k halves d-major so work starts after the kTd d=0 DMA.
                for d in range(2):
                    nc.tensor.matmul(base[:, LK:W], wq_sb[:, d, :],
                                     qTd_sb[:, d, :],
                                     start=(d == 0), stop=(d == 1))
                for d in range(2):
                    for half in range(2):
                        nc.tensor.matmul(
                            base[:, 512 * half:512 * (half + 1)],
                            wk_sb[:, d, :],
                            kTd_sb[:, d, 512 * half:512 * (half + 1)],
                            start=(d == 0), stop=(d == 1))

                def emit_fma(m):
                    p48 = p48p.tile([128, 2, W], F32, tag="p48",
                                    name=f"p48_{m}")
                    nc.vector.tensor_scalar(p48[:, 0, :], base[:],
                                            float(s_scale[m]), 48.0,
                                            MULT, ADD)
                    nc.vector.tensor_scalar(p48[:, 1, :], base[:],
                                            float(s_scale[m]), 48.0625,
                                            MULT, ADD)
                    return p48

                def emit_sin(m, p48):
                    bas = bp.tile([128, 2, W], BF16, tag="bas",
                                  name=f"bas_{m}")
                    nc.scalar.activation(bas[:],
                                         p48[:].bitcast(U16)[:, :, 0::2],
                                         SIN, scale=SCALE_SIN,
                                         bias=negpi_sb[:])
                    return bas

                def emit_scores(m, bas):
                    # qw[:,0] = cw*sin_q pairs with cos_k; qw[:,1] = cw*cos_q
                    qw = qwp.tile([128, 2, LQ], BF16, tag="qw",
                                  name=f"qw_{m}")
                    nc.vector.tensor_scalar_mul(qw[:], bas[:, :, LK:W],
                                                cw_sb[:, m:m + 1])
                    for qb in range(2):
                        for half in range(2):
                            nc.tensor.matmul(
                                sc[qb][:, 512 * half:512 * (half + 1)],
                                qw[:, 0, 128 * qb:128 * (qb + 1)],
                                bas[:, 1, 512 * half:512 * (half + 1)],
                                start=(m == 0), stop=False)
                            nc.tensor.matmul(
                                sc[qb][:, 512 * half:512 * (half + 1)],
                                qw[:, 1, 128 * qb:128 * (qb + 1)],
                                bas[:, 0, 512 * half:512 * (half + 1)],
                                start=False, stop=(m == M - 1))

                # software-pipelined: the fma pair of iteration m is emitted
                # before qw/scores of m-1 so the in-order DVE queue never
                # stalls behind the ACT sin of the previous iteration.
                prev = None
                for m in range(M):
                    p48 = emit_fma(m)
                    if prev is not None:
                        emit_scores(m - 1, prev)
                    prev = emit_sin(m, p48)
                emit_scores(M - 1, prev)

            with tc.tile_pool(name="ps_tail", bufs=2, space="PSUM") as ps_tail:
                expS = [None, None]
                for qb in range(2):
                    expS[qb] = ep.tile([128, LK], BF16, tag="exps",
                                       name=f"expS{qb}")
                    nc.scalar.activation(expS[qb][:], sc[qb][:], EXP)
                for qb in range(2):
                    expT = etp.tile([128, NKC, 128], BF16, tag="expt")
                    for c in range(NKC):
                        tp = ps_tail.tile([128, 128], BF16, tag="tp",
                                          name=f"tp{qb}{c}")
                        nc.tensor.transpose(tp[:],
                                            expS[qb][:, 128 * c:128 * (c + 1)],
                                            ident_sb[:])
                        nc.vector.tensor_copy(expT[:, c, :], tp[:])
                    av = ps_tail.tile([128, VS + 1], F32, tag="av",
                                      name=f"av{qb}")
                    for c in range(NKC):
                        nc.tensor.matmul(av[:], expT[:, c, :], vals_sb[:, c, :],
                                         start=(c == 0), stop=(c == NKC - 1))
                    r = sp.tile([128, 1], F32, tag="scal")
                    nc.vector.reciprocal(r[:], av[:, VS:VS + 1])
                    o_sb = op.tile([128, VS], F32, tag="outs")
                    nc.vector.tensor_scalar_mul(o_sb[:], av[:, 0:VS], r[:])
                    nc.sync.dma_start(out=out[qb * 128:(qb + 1) * 128, :],
                                      in_=o_sb[:])

    nc.compile()
    return nc


def _pack_rows(a):
    # [256, N] -> [128, 2, N]: row r -> (r % 128, r // 128)
    return np.ascontiguousarray(a.reshape(2, 128, -1).transpose(1, 0, 2))


def _make_in_maps(inputs) -> list[dict]:
    queries = np.asarray(inputs["queries"], dtype=np.float32)
    key = np.asarray(inputs["key"], dtype=np.float32)
    value = np.asarray(inputs["value"], dtype=np.float32)
    vl = np.asarray(inputs["valid_length"], dtype=np.int32)
    W_q = np.asarray(inputs["W_q"], dtype=np.float32)
    W_k = np.asarray(inputs["W_k"], dtype=np.float32)
    W_v = np.asarray(inputs["W_v"], dtype=np.float32)

    cfit = np.asarray(C_FIT, np.float32)
    cwm = np.ascontiguousarray((W_v[:, None] * cfit[None, :]).astype(np.float32))
    negpi = np.full((H, 1), -math.pi, np.float32)
    ident = np.eye(128, dtype=ml_dtypes.bfloat16)
    wk_b = _pack_rows(W_k.astype(ml_dtypes.bfloat16))
    wq_b = _pack_rows(W_q.astype(ml_dtypes.bfloat16))

    in_maps = []
    for b in range(B):
        v = max(int(vl[b]), 0)
        vals = np.zeros((LK, VS + 1), dtype=np.float32)
        vals[:v, :VS] = value[b, :v]
        vals[:v, VS] = 1.0
        vals = vals.astype(ml_dtypes.bfloat16)
        vals = np.ascontiguousarray(
            vals.reshape(NKC, 128, VS + 1).transpose(1, 0, 2))
        in_maps.append({
            "kTd": _pack_rows(key[b].T.astype(ml_dtypes.bfloat16)),
            "qTd": _pack_rows(queries[b].T.astype(ml_dtypes.bfloat16)),
            "wk": wk_b, "wq": wq_b, "cw": cwm,
            "negpi": negpi, "ident": ident, "vals": vals,
        })
    return in_maps


def _postprocess(res, inputs) -> np.ndarray:
    value = np.asarray(inputs["value"], dtype=np.float32)
    vl = np.asarray(inputs["valid_length"], dtype=np.int32)
    out = np.stack([np.asarray(res.results[i]["out"]) for i in range(B)], axis=0)
    for b in range(B):
        if int(vl[b]) <= 0:
            out[b] = value[b].mean(axis=0, keepdims=True)
    return out.astype(np.float32)


def kernel(**inputs) -> np.ndarray:
    if "nc" not in _CACHE:
        _CACHE["nc"] = _build()
    nc = _CACHE["nc"]
    in_maps = _make_in_maps(inputs)
    res = run_bass_kernel_spmd(nc, in_maps, core_ids=list(range(B)))
    return _postprocess(res, inputs)


# revision 16
# speedup vs baseline: 2.2649x; 1.0379x over previous
"""AdditiveAttention TRN2 kernel v7 — sin-basis scores via low-u16 binade
phase extraction.

scores[q,k] = sum_h W_v[h] tanh(qh+kh) with tanh(s) ~= sum_m c_m sin(w_m s)
factorized through sin(a+b) = sin a cos b + cos a sin b into 2M rank-128
bf16 matmuls. Per-m pipeline:

  DVE  : p48_s = (w_m/8pi)*x + 48.0     f32 in binade [32,64): the low 16
  DVE  : p48_c = (w_m/8pi)*x + 48.0625  mantissa bits ARE the phase of
                                        w_m*x in 2^16 units (+1/16 value
                                        = +pi/2 phase for the cos row)
  ACT  : bas = Sin(lo16 * 2pi/2^16 - pi) -> bf16, reading the low u16 of
         each f32 via a strided bitcast view (no mask instructions)
  DVE  : qw  = bas_q * cw[m]            (per-partition scalar multiply)
  PE   : sc += qsw^T kc + qcw^T ks      (8x 512-col bf16 matmuls)

The base projections x = [W_k^T k^T | W_q^T q^T] are computed once in
bf16 (6 matmuls) and stay in PSUM; the per-m fmas read PSUM directly.
q (256) and k (1024) columns ride together in [128, 1280]-wide ops.
Emission is software-pipelined (fma pair of iteration m before qw/scores
of m-1) so no engine stalls behind a cross-engine dependency in-order.
Dummy Exp/Sin activations at t~0 pre-load both activation tables into
the two resident slots, keeping table loads off the critical path.
"""

import math

import ml_dtypes
import numpy as np

from concourse import bacc, mybir
from concourse import tile
from concourse.bass_utils import run_bass_kernel_spmd

B, LQ, LK, QS, KS, H, VS = 8, 256, 1024, 256, 256, 128, 256
F32 = mybir.dt.float32
BF16 = mybir.dt.bfloat16

W_FIT = [0.873796, 0.28935, 1.465455, 2.101898, 3.067597]
C_FIT = [0.321106, 1.231298, 0.118058, 0.055618, 0.019224]
M = len(W_FIT)

SCALE_SIN = 2.0 * math.pi / (1 << 16)
NKC = LK // 128         # 8 key chunks of 128
W = LK + LQ             # 1280: k columns then q columns

_CACHE: dict = {}


def _build():
    nc = bacc.Bacc("TRN2", target_bir_lowering=False, debug=False)
    # packed params: fewer DMAs (each DMA costs ~620ns of queue issue time)
    qq = nc.declare_dram_parameter("qq", [128, 2, H + LQ], BF16, isOutput=False)
    kk = nc.declare_dram_parameter("kk", [128, 2, H + LK], BF16, isOutput=False)
    cn = nc.declare_dram_parameter("cn", [H, 1 + M], F32, isOutput=False)
    iv = nc.declare_dram_parameter("iv", [128, 1 + NKC, VS + 1], BF16,
                                   isOutput=False)
    out = nc.declare_dram_parameter("out", [LQ, VS], F32, isOutput=True)

    SIN = mybir.ActivationFunctionType.Sin
    EXP = mybir.ActivationFunctionType.Exp
    ADD = mybir.AluOpType.add
    MULT = mybir.AluOpType.mult
    U16 = mybir.dt.uint16

    s_scale = [w / (8.0 * math.pi) for w in W_FIT]

    with tile.TileContext(nc) as tc:
        with (
            tc.tile_pool(name="const", bufs=1) as cpool,
            tc.tile_pool(name="p48p", bufs=3) as p48p,
            tc.tile_pool(name="bp", bufs=3) as bp,
            tc.tile_pool(name="qwp", bufs=3) as qwp,
            tc.tile_pool(name="ep", bufs=2) as ep,
            tc.tile_pool(name="etp", bufs=2) as etp,
            tc.tile_pool(name="op", bufs=2) as op,
            tc.tile_pool(name="sp", bufs=2) as sp,
            tc.tile_pool(name="ps_sc", bufs=1, space="PSUM") as ps_sc,
        ):
            qq_sb = cpool.tile([128, 2, H + LQ], BF16)
            kk_sb = cpool.tile([128, 2, H + LK], BF16)
            cn_sb = cpool.tile([128, 1 + M], F32)
            iv_sb = cpool.tile([128, 1 + NKC, VS + 1], BF16)
            dumm = cpool.tile([128, 1], F32)
            dumo = cpool.tile([128, 2], F32)
            wq_sb = qq_sb[:, :, 0:H]
            qTd_sb = qq_sb[:, :, H:H + LQ]
            wk_sb = kk_sb[:, :, 0:H]
            kTd_sb = kk_sb[:, :, H:H + LK]
            negpi_sb = cn_sb[:, 0:1]
            cw_sb = cn_sb[:, 1:1 + M]
            ident_sb = iv_sb[:, 0, 0:128]
            vals_sb = iv_sb[:, 1:1 + NKC, :]

            # table pre-loads: Exp and Sin live in different act-func sets;
            # touching both on a dummy at t~0 pulls both tables into the two
            # resident slots while the DMAs run.
            nc.gpsimd.memset(dumm[:], 0.0)
            nc.scalar.activation(dumo[:, 0:1], dumm[:], EXP)
            nc.scalar.activation(dumo[:, 1:2], dumm[:], SIN)

            # critical-path DMAs on the SP queue, tail data on the Pool queue
            nc.sync.dma_start(out=qq_sb[:], in_=qq[:])
            for d in range(2):
                nc.sync.dma_start(out=kk_sb[:, d, :], in_=kk[:, d, :])
            nc.gpsimd.dma_start(out=cn_sb[:], in_=cn[:])
            nc.gpsimd.dma_start(out=iv_sb[:], in_=iv[:])

            # sc[qb]: [128, 1024] f32 = 2 PSUM banks; matmuls write 512-col
            # halves (bank-aligned), exp reads the full 1024 in one call.
            sc = [ps_sc.tile([128, LK], F32, tag=f"sc{qb}", name=f"sc{qb}")
                  for qb in range(2)]

            with tc.tile_pool(name="ps_base", bufs=1, space="PSUM") as ps_base:
                base = ps_base.tile([128, W], F32, tag="base")

                def emit_fma(m, p48, lo, hi):
                    nc.vector.tensor_scalar(p48[:, 0, lo:hi], base[:, lo:hi],
                                            float(s_scale[m]), 48.0,
                                            MULT, ADD)
                    nc.vector.tensor_scalar(p48[:, 1, lo:hi], base[:, lo:hi],
                                            float(s_scale[m]), 48.0625,
                                            MULT, ADD)

                def emit_sin(m, p48, bas, lo, hi):
                    nc.scalar.activation(
                        bas[:, :, lo:hi],
                        p48[:].bitcast(U16)[:, :, 2 * lo:2 * hi:2],
                        SIN, scale=SCALE_SIN, bias=negpi_sb)

                def emit_qw(m, bas):
                    # qw[:,0] = cw*sin_q pairs with cos_k; qw[:,1] = cw*cos_q
                    qw = qwp.tile([128, 2, LQ], BF16, tag="qw",
                                  name=f"qw_{m}")
                    nc.vector.tensor_scalar_mul(qw[:], bas[:, :, LK:W],
                                                cw_sb[:, m:m + 1])
                    return qw

                def emit_scores(m, bas, qw):
                    for qb in range(2):
                        for half in range(2):
                            nc.tensor.matmul(
                                sc[qb][:, 512 * half:512 * (half + 1)],
                                qw[:, 0, 128 * qb:128 * (qb + 1)],
                                bas[:, 1, 512 * half:512 * (half + 1)],
                                start=(m == 0), stop=False)
                            nc.tensor.matmul(
                                sc[qb][:, 512 * half:512 * (half + 1)],
                                qw[:, 1, 128 * qb:128 * (qb + 1)],
                                bas[:, 0, 512 * half:512 * (half + 1)],
                                start=False, stop=(m == M - 1))

                # base = [khT | qhT]. q projections first (their DMA is small
                # and lands first); the m=0 fma/sin/qw run on the q columns
                # while the k DMA + k projections are still in flight, so the
                # first score matmul only waits on the k-part sin.
                for d in range(2):
                    nc.tensor.matmul(base[:, LK:W], wq_sb[:, d, :],
                                     qTd_sb[:, d, :],
                                     start=(d == 0), stop=(d == 1))
                p48_0 = p48p.tile([128, 2, W], F32, tag="p48", name="p48_0")
                bas_0 = bp.tile([128, 2, W], BF16, tag="bas", name="bas_0")
                emit_fma(0, p48_0, LK, W)
                emit_sin(0, p48_0, bas_0, LK, W)
                qw_0 = emit_qw(0, bas_0)
                for d in range(2):
                    for half in range(2):
                        nc.tensor.matmul(
                            base[:, 512 * half:512 * (half + 1)],
                            wk_sb[:, d, :],
                            kTd_sb[:, d, 512 * half:512 * (half + 1)],
                            start=(d == 0), stop=(d == 1))
                emit_fma(0, p48_0, 0, LK)
                emit_sin(0, p48_0, bas_0, 0, LK)

                # software-pipelined: on the in-order DVE queue, the fma pair
                # of iteration m+1 is emitted before qw_m (which waits on the
                # ACT sin), so the fma feeding sin_{m+1} is never stuck
                # behind a cross-engine dependency.
                prev, prev_qw = bas_0, qw_0
                for m in range(1, M):
                    p48 = p48p.tile([128, 2, W], F32, tag="p48",
                                    name=f"p48_{m}")
                    emit_fma(m, p48, 0, W)
                    if m >= 2:
                        prev_qw = emit_qw(m - 1, prev)
                    emit_scores(m - 1, prev, prev_qw)
                    bas = bp.tile([128, 2, W], BF16, tag="bas",
                                  name=f"bas_{m}")
                    emit_sin(m, p48, bas, 0, W)
                    prev = bas
                prev_qw = emit_qw(M - 1, prev)
                emit_scores(M - 1, prev, prev_qw)

            with tc.tile_pool(name="ps_tail", bufs=2, space="PSUM") as ps_tail:
                expS = [None, None]
                for qb in range(2):
                    expS[qb] = ep.tile([128, LK], BF16, tag="exps",
                                       name=f"expS{qb}")
                    nc.scalar.activation(expS[qb][:], sc[qb][:], EXP)
                for qb in range(2):
                    expT = etp.tile([128, NKC, 128], BF16, tag="expt")
                    for c in range(NKC):
                        tp = ps_tail.tile([128, 128], BF16, tag="tp",
                                          name=f"tp{qb}{c}")
                        nc.tensor.transpose(tp[:],
                                            expS[qb][:, 128 * c:128 * (c + 1)],
                                            ident_sb[:])
                        nc.vector.tensor_copy(expT[:, c, :], tp[:])
                    av = ps_tail.tile([128, VS + 1], F32, tag="av",
                                      name=f"av{qb}")
                    for c in range(NKC):
                        nc.tensor.matmul(av[:], expT[:, c, :], vals_sb[:, c, :],
                                         start=(c == 0), stop=(c == NKC - 1))
                    r = sp.tile([128, 1], F32, tag="scal")
                    nc.vector.reciprocal(r[:], av[:, VS:VS + 1])
                    o_sb = op.tile([128, VS], F32, tag="outs")
                    nc.vector.tensor_scalar_mul(o_sb[:], av[:, 0:VS], r[:])
                    nc.sync.dma_start(out=out[qb * 128:(qb + 1) * 128, :],
                                      in_=o_sb[:])

    nc.compile()
    return nc


def _pack_rows(a):
    # [256, N] -> [128, 2, N]: row r -> (r % 128, r // 128)
    return np.ascontiguousarray(a.reshape(2, 128, -1).transpose(1, 0, 2))


def _make_in_maps(inputs) -> list[dict]:
    queries = np.asarray(inputs["queries"], dtype=np.float32)
    key = np.asarray(inputs["key"], dtype=np.float32)
    value = np.asarray(inputs["value"], dtype=np.float32)
    vl = np.asarray(inputs["valid_length"], dtype=np.int32)
    W_q = np.asarray(inputs["W_q"], dtype=np.float32)
    W_k = np.asarray(inputs["W_k"], dtype=np.float32)
    W_v = np.asarray(inputs["W_v"], dtype=np.float32)

    cfit = np.asarray(C_FIT, np.float32)
    cn = np.empty((H, 1 + M), np.float32)
    cn[:, 0] = -math.pi
    cn[:, 1:] = W_v[:, None] * cfit[None, :]
    cn = np.ascontiguousarray(cn)
    wk_b = _pack_rows(W_k.astype(ml_dtypes.bfloat16))
    wq_b = _pack_rows(W_q.astype(ml_dtypes.bfloat16))

    in_maps = []
    for b in range(B):
        v = max(int(vl[b]), 0)
        vals = np.zeros((LK, VS + 1), dtype=np.float32)
        vals[:v, :VS] = value[b, :v]
        vals[:v, VS] = 1.0
        iv = np.zeros((128, 1 + NKC, VS + 1), dtype=ml_dtypes.bfloat16)
        iv[:, 0, 0:128] = np.eye(128, dtype=ml_dtypes.bfloat16)
        iv[:, 1:, :] = vals.astype(ml_dtypes.bfloat16).reshape(
            NKC, 128, VS + 1).transpose(1, 0, 2)
        qq = np.concatenate(
            [wq_b, _pack_rows(queries[b].T.astype(ml_dtypes.bfloat16))],
            axis=2)
        kk = np.concatenate(
            [wk_b, _pack_rows(key[b].T.astype(ml_dtypes.bfloat16))],
            axis=2)
        in_maps.append({
            "qq": np.ascontiguousarray(qq),
            "kk": np.ascontiguousarray(kk),
            "cn": cn, "iv": np.ascontiguousarray(iv),
        })
    return in_maps


def _postprocess(res, inputs) -> np.ndarray:
    value = np.asarray(inputs["value"], dtype=np.float32)
    vl = np.asarray(inputs["valid_length"], dtype=np.int32)
    out = np.stack([np.asarray(res.results[i]["out"]) for i in range(B)], axis=0)
    for b in range(B):
        if int(vl[b]) <= 0:
            out[b] = value[b].mean(axis=0, keepdims=True)
    return out.astype(np.float32)


def kernel(**inputs) -> np.ndarray:
    if "nc" not in _CACHE:
        _CACHE["nc"] = _build()
    nc = _CACHE["nc"]
    in_maps = _make_in_maps(inputs)
    res = run_bass_kernel_spmd(nc, in_maps, core_ids=list(range(B)))
    return _postprocess(res, inputs)


# revision 20
# speedup vs baseline: 2.3890x; 1.0548x over previous
"""AdditiveAttention TRN2 kernel v7 — sin-basis scores via low-u16 binade
phase extraction.

scores[q,k] = sum_h W_v[h] tanh(qh+kh) with tanh(s) ~= sum_m c_m sin(w_m s)
factorized through sin(a+b) = sin a cos b + cos a sin b into 2M rank-128
bf16 matmuls. Per-m pipeline:

  DVE  : p48_s = (w_m/8pi)*x + 48.0     f32 in binade [32,64): the low 16
  DVE  : p48_c = (w_m/8pi)*x + 48.0625  mantissa bits ARE the phase of
                                        w_m*x in 2^16 units (+1/16 value
                                        = +pi/2 phase for the cos row)
  ACT  : bas = Sin(lo16 * 2pi/2^16 - pi) -> bf16, reading the low u16 of
         each f32 via a strided bitcast view (no mask instructions)
  DVE  : qw  = bas_q * cw[m]            (per-partition scalar multiply)
  PE   : sc += qsw^T kc + qcw^T ks      (8x 512-col bf16 matmuls)

The base projections x = [W_k^T k^T | W_q^T q^T] are computed once in
bf16 (6 matmuls) and stay in PSUM; the per-m fmas read PSUM directly.
q (256) and k (1024) columns ride together in [128, 1280]-wide ops.
Emission is software-pipelined (fma pair of iteration m before qw/scores
of m-1) so no engine stalls behind a cross-engine dependency in-order.
Dummy Exp/Sin activations at t~0 pre-load both activation tables into
the two resident slots, keeping table loads off the critical path.
"""

import math

import ml_dtypes
import numpy as np

from concourse import bacc, mybir
from concourse import tile
from concourse.bass_utils import run_bass_kernel_spmd

B, LQ, LK, QS, KS, H, VS = 8, 256, 1024, 256, 256, 128, 256
F32 = mybir.dt.float32
BF16 = mybir.dt.bfloat16

W_FIT = [0.29237, 0.87651, 1.51083, 2.50362]
C_FIT = [1.23737, 0.30825, 0.14462, 0.04779]
M = len(W_FIT)

SCALE_SIN = 2.0 * math.pi / (1 << 16)
NKC = LK // 128         # 8 key chunks of 128
W = LK + LQ             # 1280: k columns then q columns

_CACHE: dict = {}


def _build():
    nc = bacc.Bacc("TRN2", target_bir_lowering=False, debug=False)
    # packed params: fewer DMAs (each DMA costs ~620ns of queue issue time)
    qq = nc.declare_dram_parameter("qq", [128, 2, H + LQ], BF16, isOutput=False)
    kk = nc.declare_dram_parameter("kk", [128, 2, H + LK], BF16, isOutput=False)
    cn = nc.declare_dram_parameter("cn", [H, 1 + M], F32, isOutput=False)
    iv = nc.declare_dram_parameter("iv", [128, 1 + NKC, VS + 1], BF16,
                                   isOutput=False)
    out = nc.declare_dram_parameter("out", [LQ, VS], F32, isOutput=True)

    SIN = mybir.ActivationFunctionType.Sin
    EXP = mybir.ActivationFunctionType.Exp
    ADD = mybir.AluOpType.add
    MULT = mybir.AluOpType.mult
    U16 = mybir.dt.uint16

    s_scale = [w / (8.0 * math.pi) for w in W_FIT]

    with tile.TileContext(nc) as tc:
        with (
            tc.tile_pool(name="const", bufs=1) as cpool,
            tc.tile_pool(name="p48p", bufs=3) as p48p,
            tc.tile_pool(name="bp", bufs=3) as bp,
            tc.tile_pool(name="qwp", bufs=3) as qwp,
            tc.tile_pool(name="ep", bufs=2) as ep,
            tc.tile_pool(name="etp", bufs=2) as etp,
            tc.tile_pool(name="op", bufs=2) as op,
            tc.tile_pool(name="sp", bufs=2) as sp,
            tc.tile_pool(name="ps_sc", bufs=1, space="PSUM") as ps_sc,
        ):
            qq_sb = cpool.tile([128, 2, H + LQ], BF16)
            kk_sb = cpool.tile([128, 2, H + LK], BF16)
            cn_sb = cpool.tile([128, 1 + M], F32)
            iv_sb = cpool.tile([128, 1 + NKC, VS + 1], BF16)
            dumm = cpool.tile([128, 1], F32)
            dumo = cpool.tile([128, 2], F32)
            wq_sb = qq_sb[:, :, 0:H]
            qTd_sb = qq_sb[:, :, H:H + LQ]
            wk_sb = kk_sb[:, :, 0:H]
            kTd_sb = kk_sb[:, :, H:H + LK]
            negpi_sb = cn_sb[:, 0:1]
            cw_sb = cn_sb[:, 1:1 + M]
            ident_sb = iv_sb[:, 0, 0:128]
            vals_sb = iv_sb[:, 1:1 + NKC, :]

            # table pre-loads: Exp and Sin live in different act-func sets;
            # touching both on a dummy at t~0 pulls both tables into the two
            # resident slots while the DMAs run.
            nc.gpsimd.memset(dumm[:], 0.0)
            nc.scalar.activation(dumo[:, 0:1], dumm[:], EXP)
            nc.scalar.activation(dumo[:, 1:2], dumm[:], SIN)

            # critical-path DMAs on the SP queue, tail data on the Pool
            # queue. The k data is split by key-column half (each covering
            # both contraction chunks) so the half-0 projections and the
            # m=0 sin chain start while half 1 is still in flight.
            KA = H + 512
            nc.sync.dma_start(out=kk_sb[:, :, 0:KA], in_=kk[:, :, 0:KA])
            nc.sync.dma_start(out=qq_sb[:], in_=qq[:])
            nc.sync.dma_start(out=kk_sb[:, :, KA:], in_=kk[:, :, KA:])
            nc.gpsimd.dma_start(out=cn_sb[:], in_=cn[:])
            nc.gpsimd.dma_start(out=iv_sb[:], in_=iv[:])

            # sc[qb]: [128, 1024] f32 = 2 PSUM banks; matmuls write 512-col
            # halves (bank-aligned), exp reads the full 1024 in one call.
            sc = [ps_sc.tile([128, LK], F32, tag=f"sc{qb}", name=f"sc{qb}")
                  for qb in range(2)]

            with tc.tile_pool(name="ps_base", bufs=1, space="PSUM") as ps_base:
                base = ps_base.tile([128, W], F32, tag="base")

                def emit_fma(m, p48, lo, hi):
                    nc.vector.tensor_scalar(p48[:, 0, lo:hi], base[:, lo:hi],
                                            float(s_scale[m]), 48.0,
                                            MULT, ADD)
                    nc.vector.tensor_scalar(p48[:, 1, lo:hi], base[:, lo:hi],
                                            float(s_scale[m]), 48.0625,
                                            MULT, ADD)

                def emit_sin(m, p48, bas, lo, hi):
                    nc.scalar.activation(
                        bas[:, :, lo:hi],
                        p48[:].bitcast(U16)[:, :, 2 * lo:2 * hi:2],
                        SIN, scale=SCALE_SIN, bias=negpi_sb)

                def emit_qw(m, bas):
                    # qw[:,0] = cw*sin_q pairs with cos_k; qw[:,1] = cw*cos_q
                    qw = qwp.tile([128, 2, LQ], BF16, tag="qw",
                                  name=f"qw_{m}")
                    nc.vector.tensor_scalar_mul(qw[:], bas[:, :, LK:W],
                                                cw_sb[:, m:m + 1])
                    return qw

                def emit_scores(m, bas, qw):
                    for qb in range(2):
                        for half in range(2):
                            nc.tensor.matmul(
                                sc[qb][:, 512 * half:512 * (half + 1)],
                                qw[:, 0, 128 * qb:128 * (qb + 1)],
                                bas[:, 1, 512 * half:512 * (half + 1)],
                                start=(m == 0), stop=False)
                            nc.tensor.matmul(
                                sc[qb][:, 512 * half:512 * (half + 1)],
                                qw[:, 1, 128 * qb:128 * (qb + 1)],
                                bas[:, 0, 512 * half:512 * (half + 1)],
                                start=False, stop=(m == M - 1))

                def emit_scores0_half(half, bas, qw):
                    for qb in range(2):
                        nc.tensor.matmul(
                            sc[qb][:, 512 * half:512 * (half + 1)],
                            qw[:, 0, 128 * qb:128 * (qb + 1)],
                            bas[:, 1, 512 * half:512 * (half + 1)],
                            start=True, stop=False)
                        nc.tensor.matmul(
                            sc[qb][:, 512 * half:512 * (half + 1)],
                            qw[:, 1, 128 * qb:128 * (qb + 1)],
                            bas[:, 0, 512 * half:512 * (half + 1)],
                            start=False, stop=False)

                # base = [khT | qhT]. PE order follows DMA arrival: k half 0,
                # then q, then k half 1. The m=0 fma/sin chain is split per
                # region so the first score matmuls only wait on the half-0
                # sin while half 1 is still in the DMA.
                p48_0 = p48p.tile([128, 2, W], F32, tag="p48", name="p48_0")
                bas_0 = bp.tile([128, 2, W], BF16, tag="bas", name="bas_0")
                for d in range(2):
                    nc.tensor.matmul(base[:, 0:512], wk_sb[:, d, :],
                                     kTd_sb[:, d, 0:512],
                                     start=(d == 0), stop=(d == 1))
                for d in range(2):
                    nc.tensor.matmul(base[:, LK:W], wq_sb[:, d, :],
                                     qTd_sb[:, d, :],
                                     start=(d == 0), stop=(d == 1))
                emit_fma(0, p48_0, LK, W)     # q part
                emit_sin(0, p48_0, bas_0, LK, W)
                emit_fma(0, p48_0, 0, 512)    # k half 0
                emit_sin(0, p48_0, bas_0, 0, 512)
                qw_0 = emit_qw(0, bas_0)
                for d in range(2):
                    nc.tensor.matmul(base[:, 512:1024], wk_sb[:, d, :],
                                     kTd_sb[:, d, 512:1024],
                                     start=(d == 0), stop=(d == 1))
                emit_scores0_half(0, bas_0, qw_0)
                emit_fma(0, p48_0, 512, 1024)  # k half 1
                emit_sin(0, p48_0, bas_0, 512, 1024)
                emit_scores0_half(1, bas_0, qw_0)

                # software-pipelined: on the in-order DVE queue, the fma pair
                # of iteration m+1 is emitted before qw_m (which waits on the
                # ACT sin), so the fma feeding sin_{m+1} is never stuck
                # behind a cross-engine dependency.
                prev, prev_qw = bas_0, qw_0
                for m in range(1, M):
                    p48 = p48p.tile([128, 2, W], F32, tag="p48",
                                    name=f"p48_{m}")
                    emit_fma(m, p48, 0, W)
                    if m >= 2:
                        prev_qw = emit_qw(m - 1, prev)
                    emit_scores(m - 1, prev, prev_qw)
                    bas = bp.tile([128, 2, W], BF16, tag="bas",
                                  name=f"bas_{m}")
                    emit_sin(m, p48, bas, 0, W)
                    prev = bas
                prev_qw = emit_qw(M - 1, prev)
                emit_scores(M - 1, prev, prev_qw)

            with tc.tile_pool(name="ps_tail", bufs=2, space="PSUM") as ps_tail:
                expS = [None, None]
                for qb in range(2):
                    expS[qb] = ep.tile([128, LK], BF16, tag="exps",
                                       name=f"expS{qb}")
                    nc.scalar.activation(expS[qb][:], sc[qb][:], EXP)
                for qb in range(2):
                    expT = etp.tile([128, NKC, 128], BF16, tag="expt")
                    for c in range(NKC):
                        tp = ps_tail.tile([128, 128], BF16, tag="tp",
                                          name=f"tp{qb}{c}")
                        nc.tensor.transpose(tp[:],
                                            expS[qb][:, 128 * c:128 * (c + 1)],
                                            ident_sb[:])
                        nc.vector.tensor_copy(expT[:, c, :], tp[:])
                    av = ps_tail.tile([128, VS + 1], F32, tag="av",
                                      name=f"av{qb}")
                    for c in range(NKC):
                        nc.tensor.matmul(av[:], expT[:, c, :], vals_sb[:, c, :],
                                         start=(c == 0), stop=(c == NKC - 1))
                    r = sp.tile([128, 1], F32, tag="scal")
                    nc.vector.reciprocal(r[:], av[:, VS:VS + 1])
                    o_sb = op.tile([128, VS], F32, tag="outs")
                    nc.vector.tensor_scalar_mul(o_sb[:], av[:, 0:VS], r[:])
                    nc.sync.dma_start(out=out[qb * 128:(qb + 1) * 128, :],
                                      in_=o_sb[:])

    nc.compile()
    return nc


def _pack_rows(a):
    # [256, N] -> [128, 2, N]: row r -> (r % 128, r // 128)
    return np.ascontiguousarray(a.reshape(2, 128, -1).transpose(1, 0, 2))


def _make_in_maps(inputs) -> list[dict]:
    queries = np.asarray(inputs["queries"], dtype=np.float32)
    key = np.asarray(inputs["key"], dtype=np.float32)
    value = np.asarray(inputs["value"], dtype=np.float32)
    vl = np.asarray(inputs["valid_length"], dtype=np.int32)
    W_q = np.asarray(inputs["W_q"], dtype=np.float32)
    W_k = np.asarray(inputs["W_k"], dtype=np.float32)
    W_v = np.asarray(inputs["W_v"], dtype=np.float32)

    cfit = np.asarray(C_FIT, np.float32)
    cn = np.empty((H, 1 + M), np.float32)
    cn[:, 0] = -math.pi
    cn[:, 1:] = W_v[:, None] * cfit[None, :]
    cn = np.ascontiguousarray(cn)
    wk_b = _pack_rows(W_k.astype(ml_dtypes.bfloat16))
    wq_b = _pack_rows(W_q.astype(ml_dtypes.bfloat16))

    in_maps = []
    for b in range(B):
        v = max(int(vl[b]), 0)
        vals = np.zeros((LK, VS + 1), dtype=np.float32)
        vals[:v, :VS] = value[b, :v]
        vals[:v, VS] = 1.0
        iv = np.zeros((128, 1 + NKC, VS + 1), dtype=ml_dtypes.bfloat16)
        iv[:, 0, 0:128] = np.eye(128, dtype=ml_dtypes.bfloat16)
        iv[:, 1:, :] = vals.astype(ml_dtypes.bfloat16).reshape(
            NKC, 128, VS + 1).transpose(1, 0, 2)
        qq = np.concatenate(
            [wq_b, _pack_rows(queries[b].T.astype(ml_dtypes.bfloat16))],
            axis=2)
        kk = np.concatenate(
            [wk_b, _pack_rows(key[b].T.astype(ml_dtypes.bfloat16))],
            axis=2)
        in_maps.append({
            "qq": np.ascontiguousarray(qq),
            "kk": np.ascontiguousarray(kk),
            "cn": cn, "iv": np.ascontiguousarray(iv),
        })
    return in_maps


def _postprocess(res, inputs) -> np.ndarray:
    value = np.asarray(inputs["value"], dtype=np.float32)
    vl = np.asarray(inputs["valid_length"], dtype=np.int32)
    out = np.stack([np.asarray(res.results[i]["out"]) for i in range(B)], axis=0)
    for b in range(B):
        if int(vl[b]) <= 0:
            out[b] = value[b].mean(axis=0, keepdims=True)
    return out.astype(np.float32)


def kernel(**inputs) -> np.ndarray:
    if "nc" not in _CACHE:
        _CACHE["nc"] = _build()
    nc = _CACHE["nc"]
    in_maps = _make_in_maps(inputs)
    res = run_bass_kernel_spmd(nc, in_maps, core_ids=list(range(B)))
    return _postprocess(res, inputs)


# revision 30
# speedup vs baseline: 2.4339x; 1.0188x over previous
"""AdditiveAttention TRN2 kernel v7 — sin-basis scores via low-u16 binade
phase extraction.

scores[q,k] = sum_h W_v[h] tanh(qh+kh) with tanh(s) ~= sum_m c_m sin(w_m s)
factorized through sin(a+b) = sin a cos b + cos a sin b into 2M rank-128
bf16 matmuls. Per-m pipeline:

  DVE  : p48_s = (w_m/8pi)*x + 48.0     f32 in binade [32,64): the low 16
  DVE  : p48_c = (w_m/8pi)*x + 48.0625  mantissa bits ARE the phase of
                                        w_m*x in 2^16 units (+1/16 value
                                        = +pi/2 phase for the cos row)
  ACT  : bas = Sin(lo16 * 2pi/2^16 - pi) -> bf16, reading the low u16 of
         each f32 via a strided bitcast view (no mask instructions)
  DVE  : qw  = bas_q * cw[m]            (per-partition scalar multiply)
  PE   : sc += qsw^T kc + qcw^T ks      (8x 512-col bf16 matmuls)

The base projections x = [W_k^T k^T | W_q^T q^T] are computed once in
bf16 (6 matmuls) and stay in PSUM; the per-m fmas read PSUM directly.
q (256) and k (1024) columns ride together in [128, 1280]-wide ops.
Emission is software-pipelined (fma pair of iteration m before qw/scores
of m-1) so no engine stalls behind a cross-engine dependency in-order.
Dummy Exp/Sin activations at t~0 pre-load both activation tables into
the two resident slots, keeping table loads off the critical path.
"""

import math

import ml_dtypes
import numpy as np

from concourse import bacc, mybir
from concourse import tile
from concourse.bass_utils import run_bass_kernel_spmd

B, LQ, LK, QS, KS, H, VS = 8, 256, 1024, 256, 256, 128, 256
F32 = mybir.dt.float32
BF16 = mybir.dt.bfloat16

W_FIT = [0.29237, 0.87651, 1.51083, 2.50362]
C_FIT = [1.23737, 0.30825, 0.14462, 0.04779]
M = len(W_FIT)

SCALE_SIN = 2.0 * math.pi / (1 << 16)
NKC = LK // 128         # 8 key chunks of 128
W = LK + LQ             # 1280: k columns then q columns

_CACHE: dict = {}


def _build():
    nc = bacc.Bacc("TRN2", target_bir_lowering=False, debug=False)
    # packed params: fewer DMAs (each DMA costs ~620ns of queue issue time)
    qq = nc.declare_dram_parameter("qq", [128, 2, H + LQ], BF16, isOutput=False)
    kk = nc.declare_dram_parameter("kk", [128, 2, H + LK], BF16, isOutput=False)
    cn = nc.declare_dram_parameter("cn", [H, 1 + M], F32, isOutput=False)
    iv = nc.declare_dram_parameter("iv", [128, 1 + NKC, VS + 1], BF16,
                                   isOutput=False)
    # av rows: [sum_k attn*v | sum_k attn]; the division happens on host
    out = nc.declare_dram_parameter("out", [LQ, VS + 1], F32, isOutput=True)

    SIN = mybir.ActivationFunctionType.Sin
    EXP = mybir.ActivationFunctionType.Exp
    ADD = mybir.AluOpType.add
    MULT = mybir.AluOpType.mult
    U16 = mybir.dt.uint16

    s_scale = [w / (8.0 * math.pi) for w in W_FIT]

    with tile.TileContext(nc) as tc:
        with (
            tc.tile_pool(name="const", bufs=1) as cpool,
            tc.tile_pool(name="p48p", bufs=3) as p48p,
            tc.tile_pool(name="bp", bufs=3) as bp,
            tc.tile_pool(name="qwp", bufs=3) as qwp,
            tc.tile_pool(name="ep", bufs=2) as ep,
            tc.tile_pool(name="etp", bufs=2) as etp,
            tc.tile_pool(name="ps_sc", bufs=1, space="PSUM") as ps_sc,
        ):
            qq_sb = cpool.tile([128, 2, H + LQ], BF16)
            kk_sb = cpool.tile([128, 2, H + LK], BF16)
            cn_sb = cpool.tile([128, 1 + M], F32)
            iv_sb = cpool.tile([128, 1 + NKC, VS + 1], BF16)
            base_sb = cpool.tile([128, W], F32)
            dumm = cpool.tile([128, 1], F32)
            dumo = cpool.tile([128, 2], F32)
            wq_sb = qq_sb[:, :, 0:H]
            qTd_sb = qq_sb[:, :, H:H + LQ]
            wk_sb = kk_sb[:, :, 0:H]
            kTd_sb = kk_sb[:, :, H:H + LK]
            negpi_sb = cn_sb[:, 0:1]
            cw_sb = cn_sb[:, 1:1 + M]
            ident_sb = iv_sb[:, 0, 0:128]
            vals_sb = iv_sb[:, 1:1 + NKC, :]

            # table pre-loads: Exp and Sin live in different act-func sets;
            # touching both on a dummy at t~0 pulls both tables into the two
            # resident slots while the DMAs run.
            nc.gpsimd.memset(dumm[:], 0.0)
            nc.scalar.activation(dumo[:, 0:1], dumm[:], EXP)
            nc.scalar.activation(dumo[:, 1:2], dumm[:], SIN)

            # critical-path DMAs on the SP queue, tail data on the Pool
            # queue. The k data is split by key-column half (each covering
            # both contraction chunks) so the half-0 projections and the
            # m=0 sin chain start while half 1 is still in flight.
            KA = H + 512
            nc.sync.dma_start(out=kk_sb[:, :, 0:KA], in_=kk[:, :, 0:KA])
            nc.sync.dma_start(out=qq_sb[:], in_=qq[:])
            nc.sync.dma_start(out=kk_sb[:, :, KA:], in_=kk[:, :, KA:])
            nc.gpsimd.dma_start(out=cn_sb[:], in_=cn[:])
            nc.gpsimd.dma_start(out=iv_sb[:], in_=iv[:])

            # sc[qb]: [128, 1024] f32 = 2 PSUM banks; matmuls write 512-col
            # halves (bank-aligned), exp reads the full 1024 in one call.
            sc = [ps_sc.tile([128, LK], F32, tag=f"sc{qb}", name=f"sc{qb}")
                  for qb in range(2)]

            with tc.tile_pool(name="ps_base", bufs=1, space="PSUM") as ps_base:
                base = ps_base.tile([128, W], F32, tag="base")

                def emit_fma(m, p48, lo, hi, src=None):
                    src = base if src is None else src
                    nc.vector.tensor_scalar(p48[:, 0, lo:hi], src[:, lo:hi],
                                            float(s_scale[m]), 48.0,
                                            MULT, ADD)
                    nc.vector.tensor_scalar(p48[:, 1, lo:hi], src[:, lo:hi],
                                            float(s_scale[m]), 48.0625,
                                            MULT, ADD)

                def emit_sin(m, p48, bas, lo, hi):
                    nc.scalar.activation(
                        bas[:, :, lo:hi],
                        p48[:].bitcast(U16)[:, :, 2 * lo:2 * hi:2],
                        SIN, scale=SCALE_SIN, bias=negpi_sb)

                def emit_qw(m, bas):
                    # qw[:,0] = cw*sin_q pairs with cos_k; qw[:,1] = cw*cos_q
                    qw = qwp.tile([128, 2, LQ], BF16, tag="qw",
                                  name=f"qw_{m}")
                    nc.vector.tensor_scalar_mul(qw[:], bas[:, :, LK:W],
                                                cw_sb[:, m:m + 1])
                    return qw

                def emit_scores(m, bas, qw):
                    for qb in range(2):
                        for half in range(2):
                            nc.tensor.matmul(
                                sc[qb][:, 512 * half:512 * (half + 1)],
                                qw[:, 0, 128 * qb:128 * (qb + 1)],
                                bas[:, 1, 512 * half:512 * (half + 1)],
                                start=(m == 0), stop=False)
                            nc.tensor.matmul(
                                sc[qb][:, 512 * half:512 * (half + 1)],
                                qw[:, 1, 128 * qb:128 * (qb + 1)],
                                bas[:, 0, 512 * half:512 * (half + 1)],
                                start=False, stop=(m == M - 1))

                def emit_scores0_half(half, bas, qw):
                    for qb in range(2):
                        nc.tensor.matmul(
                            sc[qb][:, 512 * half:512 * (half + 1)],
                            qw[:, 0, 128 * qb:128 * (qb + 1)],
                            bas[:, 1, 512 * half:512 * (half + 1)],
                            start=True, stop=False)
                        nc.tensor.matmul(
                            sc[qb][:, 512 * half:512 * (half + 1)],
                            qw[:, 1, 128 * qb:128 * (qb + 1)],
                            bas[:, 0, 512 * half:512 * (half + 1)],
                            start=False, stop=False)

                # base = [khT | qhT]. PE order follows DMA arrival: k half 0,
                # then q, then k half 1. The m=0 fma/sin chain is split per
                # region so the first score matmuls only wait on the half-0
                # sin while half 1 is still in the DMA.
                p48_0 = p48p.tile([128, 2, W], F32, tag="p48", name="p48_0")
                bas_0 = bp.tile([128, 2, W], BF16, tag="bas", name="bas_0")
                for d in range(2):
                    nc.tensor.matmul(base[:, 0:512], wk_sb[:, d, :],
                                     kTd_sb[:, d, 0:512],
                                     start=(d == 0), stop=(d == 1))
                for d in range(2):
                    nc.tensor.matmul(base[:, LK:W], wq_sb[:, d, :],
                                     qTd_sb[:, d, :],
                                     start=(d == 0), stop=(d == 1))
                emit_fma(0, p48_0, LK, W)     # q part
                emit_sin(0, p48_0, bas_0, LK, W)
                emit_fma(0, p48_0, 0, 512)    # k half 0
                emit_sin(0, p48_0, bas_0, 0, 512)
                # PSUM->SBUF base copy in DVE slack: DVE reads PSUM at half
                # throughput, so the 2(M-1) steady-state fmas read SBUF.
                nc.vector.tensor_copy(base_sb[:, LK:W], base[:, LK:W])
                nc.vector.tensor_copy(base_sb[:, 0:512], base[:, 0:512])
                qw_0 = emit_qw(0, bas_0)
                for d in range(2):
                    nc.tensor.matmul(base[:, 512:1024], wk_sb[:, d, :],
                                     kTd_sb[:, d, 512:1024],
                                     start=(d == 0), stop=(d == 1))
                emit_scores0_half(0, bas_0, qw_0)
                emit_fma(0, p48_0, 512, 1024)  # k half 1
                emit_sin(0, p48_0, bas_0, 512, 1024)
                nc.vector.tensor_copy(base_sb[:, 512:1024], base[:, 512:1024])
                emit_scores0_half(1, bas_0, qw_0)

                # software-pipelined: on the in-order DVE queue, the fma pair
                # of iteration m+1 is emitted before qw_m (which waits on the
                # ACT sin), so the fma feeding sin_{m+1} is never stuck
                # behind a cross-engine dependency.
                prev, prev_qw = bas_0, qw_0
                for m in range(1, M):
                    p48 = p48p.tile([128, 2, W], F32, tag="p48",
                                    name=f"p48_{m}")
                    emit_fma(m, p48, 0, W, src=base_sb)
                    if m >= 2:
                        prev_qw = emit_qw(m - 1, prev)
                    emit_scores(m - 1, prev, prev_qw)
                    bas = bp.tile([128, 2, W], BF16, tag="bas",
                                  name=f"bas_{m}")
                    emit_sin(m, p48, bas, 0, W)
                    prev = bas
                prev_qw = emit_qw(M - 1, prev)
                emit_scores(M - 1, prev, prev_qw)

            with tc.tile_pool(name="ps_tail", bufs=2, space="PSUM") as ps_tail:
                expS = [None, None]
                for qb in range(2):
                    expS[qb] = ep.tile([128, LK], BF16, tag="exps",
                                       name=f"expS{qb}")
                    nc.scalar.activation(expS[qb][:], sc[qb][:], EXP)
                for qb in range(2):
                    expT = etp.tile([128, NKC, 128], BF16, tag="expt")
                    for c in range(NKC):
                        tp = ps_tail.tile([128, 128], BF16, tag="tp",
                                          name=f"tp{qb}{c}")
                        nc.tensor.transpose(tp[:],
                                            expS[qb][:, 128 * c:128 * (c + 1)],
                                            ident_sb[:])
                        nc.vector.tensor_copy(expT[:, c, :], tp[:])
                    av = ps_tail.tile([128, VS + 1], F32, tag="av",
                                      name=f"av{qb}")
                    for c in range(NKC):
                        nc.tensor.matmul(av[:], expT[:, c, :], vals_sb[:, c, :],
                                         start=(c == 0), stop=(c == NKC - 1))
                    # numerator and denominator ship together; host divides.
                    o_sb = ep.tile([128, VS + 1], F32, tag="osb",
                                   name=f"osb{qb}")
                    nc.vector.tensor_copy(o_sb[:], av[:])
                    eng = nc.sync if qb == 0 else nc.gpsimd
                    eng.dma_start(out=out[qb * 128:(qb + 1) * 128, :],
                                  in_=o_sb[:])

    nc.compile()
    return nc


def _pack_rows(a):
    # [256, N] -> [128, 2, N]: row r -> (r % 128, r // 128)
    return np.ascontiguousarray(a.reshape(2, 128, -1).transpose(1, 0, 2))


def _make_in_maps(inputs) -> list[dict]:
    queries = np.asarray(inputs["queries"], dtype=np.float32)
    key = np.asarray(inputs["key"], dtype=np.float32)
    value = np.asarray(inputs["value"], dtype=np.float32)
    vl = np.asarray(inputs["valid_length"], dtype=np.int32)
    W_q = np.asarray(inputs["W_q"], dtype=np.float32)
    W_k = np.asarray(inputs["W_k"], dtype=np.float32)
    W_v = np.asarray(inputs["W_v"], dtype=np.float32)

    cfit = np.asarray(C_FIT, np.float32)
    cn = np.empty((H, 1 + M), np.float32)
    cn[:, 0] = -math.pi
    cn[:, 1:] = W_v[:, None] * cfit[None, :]
    cn = np.ascontiguousarray(cn)
    wk_b = _pack_rows(W_k.astype(ml_dtypes.bfloat16))
    wq_b = _pack_rows(W_q.astype(ml_dtypes.bfloat16))

    in_maps = []
    for b in range(B):
        v = max(int(vl[b]), 0)
        vals = np.zeros((LK, VS + 1), dtype=np.float32)
        vals[:v, :VS] = value[b, :v]
        vals[:v, VS] = 1.0
        iv = np.zeros((128, 1 + NKC, VS + 1), dtype=ml_dtypes.bfloat16)
        iv[:, 0, 0:128] = np.eye(128, dtype=ml_dtypes.bfloat16)
        iv[:, 1:, :] = vals.astype(ml_dtypes.bfloat16).reshape(
            NKC, 128, VS + 1).transpose(1, 0, 2)
        qq = np.concatenate(
            [wq_b, _pack_rows(queries[b].T.astype(ml_dtypes.bfloat16))],
            axis=2)
        kk = np.concatenate(
            [wk_b, _pack_rows(key[b].T.astype(ml_dtypes.bfloat16))],
            axis=2)
        in_maps.append({
            "qq": np.ascontiguousarray(qq),
            "kk": np.ascontiguousarray(kk),
            "cn": cn, "iv": np.ascontiguousarray(iv),
        })
    return in_maps


def _postprocess(res, inputs) -> np.ndarray:
    value = np.asarray(inputs["value"], dtype=np.float32)
    vl = np.asarray(inputs["valid_length"], dtype=np.int32)
    av = np.stack([np.asarray(res.results[i]["out"]) for i in range(B)], axis=0)
    with np.errstate(divide="ignore", invalid="ignore"):
        out = av[:, :, :VS] / av[:, :, VS:VS + 1]
    for b in range(B):
        if int(vl[b]) <= 0:
            out[b] = value[b].mean(axis=0, keepdims=True)
    return out.astype(np.float32)


def kernel(**inputs) -> np.ndarray:
    if "nc" not in _CACHE:
        _CACHE["nc"] = _build()
    nc = _CACHE["nc"]
    in_maps = _make_in_maps(inputs)
    res = run_bass_kernel_spmd(nc, in_maps, core_ids=list(range(B)))
    return _postprocess(res, inputs)
